# revision 1
# baseline (speedup 1.0000x reference)
"""Trainium2 Bass kernel for CoreSageLayer (GNN mean-aggregate + 3-way linear).

Computation (reference):
    mask = (adj == 1)                      # [N, N] 0/1
    deg  = mask.sum(axis=1)                # [N]
    x1   = (mask @ x) / deg[:, None]       # [N, F]
    out[k] = concat([x1, x], 1) @ W[k] + bias   # [3, N, O]

Distribution: row-shard adj / x1 / out over nodes across 8 cores; replicate
x and weights; no collectives (rows independent).

Device schedule per core (nodes NB=1024, 8 node-tiles of 128):
  stage 1 (per node-tile j): PSUM[128, 257] accumulates 64 matmuls
      lhsT = adjT chunk [128 m, 128 n] (host pre-transposed, cast to bf16),
      rhs  = [x | ones] chunk [128 m, 257]  -> col 256 = degree (exact: 0/1
      products accumulate in fp32 PSUM).
  finalize j: rec = 1/deg (DVE), x1 = psum * rec (DVE), PE-transpose x1 into
      x1T, then stage 2: out[k, j] = [x1, x]^T-contracted matmuls in fp32.
"""

import sys

sys.path.insert(0, "/opt/trn_rl_repo")

import numpy as np

N = 8192
F = 256
O = 256
NCORES = 8
NB = N // NCORES          # nodes per core (1024)
JT = NB // 128            # node tiles per core (8)
MCHUNKS = N // 128        # contraction chunks (64)
FP = F + 1                # x columns + ones column (257)

_MASK_BF16 = True         # adjacency + x in bf16 for stage 1 (mask exact in bf16)


def _patch_tile_drain():
    """This container's walrus allows only one sync-wait per CTRL instruction;
    split the Tile kernel-tail drain's waits onto single-wait no-fuse NoOps."""
    import concourse.tile as tile
    from concourse import mybir
    from concourse.tile import ScopedClock

    if getattr(tile.TileContext, "_drain_split_patched", False):
        return

    def _drain_and_barrier(self, tick_clock, wait_clock):
        nc = self.nc
        drain_inst = nc.sync.drain()
        wait_clock.add_sem_waits(
            drain_inst.ins, ScopedClock({None: tick_clock.global_clock})
        )
        si = drain_inst.ins.sync_info
        if si is not None and len(si.on_wait) > 1:
            waits = list(si.on_wait)
            drain_inst.ins.sync_info = mybir.SyncInfo(
                on_wait=[waits[0]], on_update=list(si.on_update)
            )
            for w in waits[1:]:
                nop = nc.sync.nop(nofuse=True, hint="split_wait")
                nop.ins.sync_info = mybir.SyncInfo(on_wait=[w], on_update=[])
        nc.all_engine_barrier()
        assert self.sems is not None
        popped = nc._tile_sem_poison_stack.pop()
        assert popped is self._sem_poison
        nc.clear_and_free_semaphores(list(self.sems.allocated().values()))
        nc.all_engine_barrier()

    tile.TileContext._drain_and_barrier = _drain_and_barrier
    tile.TileContext._drain_split_patched = True

    # Same walrus limitation, general case: any instruction that Tile gave
    # >1 sem-wait (e.g. a DMA with both RAW and WAR deps) fails codegen.
    # Split surplus waits onto fresh single-wait NoOps emitted just before
    # the instruction on the same engine, at the serialized-BIR level.
    import concourse.bass as bass
    import orjson

    _orig_to_json_bytes = bass.Bass.to_json_bytes

    def _to_json_bytes_split(self):
        m = orjson.loads(_orig_to_json_bytes(self))
        ctr = 0
        for fn in m.get("functions", []):
            for bb in fn.get("blocks", []):
                insts = bb.get("instructions", [])
                new = []
                for inst in insts:
                    si = inst.get("sync_info")
                    waits = (si or {}).get("on_wait") or []
                    if len(waits) > 1:
                        for w in waits[:-1]:
                            ctr += 1
                            new.append({
                                "name": f"SWNOP-{ctr}",
                                "opcode": "NoOp",
                                "engine": inst["engine"],
                                "ins": [],
                                "outs": [],
                                "sync_info": {"on_wait": [w], "on_update": []},
                            })
                        si["on_wait"] = [waits[-1]]
                    new.append(inst)
                bb["instructions"] = new
        return orjson.dumps(m)

    bass.Bass.to_json_bytes = _to_json_bytes_split


def build_bass(with_bias: bool):
    import concourse.bass as bass
    import concourse.tile as tile
    from concourse import mybir
    from concourse.masks import make_identity

    _patch_tile_drain()

    mask_dt = mybir.dt.bfloat16 if _MASK_BF16 else mybir.dt.float32
    f32 = mybir.dt.float32

    nc = bass.Bass()
    maskt = nc.dram_tensor("maskt", [JT, 128, MCHUNKS * 128], mask_dt,
                           kind="ExternalInput")
    xp = nc.dram_tensor("xp", [128, MCHUNKS * FP], mask_dt, kind="ExternalInput")
    f32r = mybir.dt.float32r
    # stage-2 operands live as float32r (same bits as fp32 in DRAM/SBUF;
    # the PE runs fp32r matmuls at 4x the fp32 rate for free dim >= 256)
    xt = nc.dram_tensor("xt", [F // 128, 128, NB], f32r, kind="ExternalInput")
    w = nc.dram_tensor("w", [3, 2 * F // 128, 128, O], f32r, kind="ExternalInput")
    if with_bias:
        biasr = nc.dram_tensor("biasr", [128, O], f32, kind="ExternalInput")
    out = nc.dram_tensor("out", [3, NB, O], f32, kind="ExternalOutput")

    FCH = 2 * F // 128  # 4 f-chunks of 128 in the stage-2 contraction

    with tile.TileContext(nc) as tc:
        with (
            tc.tile_pool(name="const", bufs=1) as const_pool,
            tc.tile_pool(name="mask", bufs=3) as mask_pool,
            tc.tile_pool(name="work", bufs=3) as work_pool,
            tc.tile_pool(name="psum1", bufs=2, space="PSUM") as psum1_pool,
            tc.tile_pool(name="psumt", bufs=2, space="PSUM") as psumt_pool,
            tc.tile_pool(name="psum2", bufs=2, space="PSUM") as psum2_pool,
        ):
            # ---- DMA order matters: everything shares the HWDGE FIFO.
            # First the j=0 mask block (split so PE can start after the first
            # sixteenth), then x|1 (split; chunk c only needs slice c//8),
            # then weights/xT (not needed until finalize(0)).
            mt0 = mask_pool.tile([128, MCHUNKS * 128], mask_dt, tag="mt", name="mt0")
            xp_sb = const_pool.tile([128, MCHUNKS * FP], mask_dt)
            # interleave the j=0 mask quarters with the x|1 eighths so chunk c
            # (needs mask piece c//16 and xp piece c//8) unblocks at stream rate
            MSPL, XSPL = 4, 8
            mw = MCHUNKS * 128 // MSPL
            xw = MCHUNKS * FP // XSPL
            for q in range(MSPL):
                nc.sync.dma_start(mt0[:, q * mw:(q + 1) * mw],
                                  maskt[0, :, q * mw:(q + 1) * mw])
                for xq in (2 * q, 2 * q + 1):
                    nc.sync.dma_start(xp_sb[:, xq * xw:(xq + 1) * xw],
                                      xp[:, xq * xw:(xq + 1) * xw])

            def stage1(j, mt):
                ps = psum1_pool.tile([128, FP], f32, tag="acc")
                for c in range(MCHUNKS):
                    nc.tensor.matmul(
                        ps[:],
                        mt[:, c * 128:(c + 1) * 128],
                        xp_sb[:, c * FP:(c + 1) * FP],
                        start=(c == 0),
                        stop=(c == MCHUNKS - 1),
                    )
                return ps

            ps0 = stage1(0, mt0)

            # stage-2 constants: emitted after stage1(0), used by finalize(0)
            xt_sb = [const_pool.tile([128, NB], f32r, tag=f"xt{h}", name=f"xt{h}")
                     for h in range(2)]
            for h in range(2):
                nc.scalar.dma_start(xt_sb[h][:], xt[h])
            w_sb = [
                [const_pool.tile([128, O], f32r, tag=f"w{k}_{fc}", name=f"w{k}_{fc}")
                 for fc in range(FCH)]
                for k in range(3)
            ]
            for k in range(3):
                for fc in range(FCH):
                    nc.scalar.dma_start(w_sb[k][fc][:], w[k, fc])
            if with_bias:
                bias_sb = const_pool.tile([128, O], f32)
                nc.scalar.dma_start(bias_sb[:], biasr[:])
            identity = const_pool.tile([128, 128], f32)
            make_identity(nc, identity)
            # x1T[h] row f (= h*128+f), col n: x1 transposed, filled per j
            x1t_sb = [const_pool.tile([128, NB], f32r, tag=f"x1t{h}", name=f"x1t{h}")
                      for h in range(2)]

            def load_mask(j):
                mt = mask_pool.tile([128, MCHUNKS * 128], mask_dt, tag="mt",
                                    name=f"mt{j}")
                nc.sync.dma_start(mt[:], maskt[j])
                return mt

            def finalize(j, ps):
                jcols = slice(j * 128, (j + 1) * 128)
                rec = work_pool.tile([128, 1], f32, tag="rec")
                nc.vector.reciprocal(rec[:], ps[:, F:F + 1])
                x1 = work_pool.tile([128, F], f32, tag="x1")
                nc.vector.tensor_scalar_mul(x1[:], ps[:, 0:F], rec[:])
                for h in range(2):
                    pt = psumt_pool.tile([128, 128], f32, tag="pt")
                    nc.tensor.transpose(pt[:], x1[:, h * 128:(h + 1) * 128], identity[:])
                    nc.vector.tensor_copy(x1t_sb[h][:, jcols], pt[:])
                # stage-2 contraction in float32r: 4x the fp32 matmul rate at
                # near-fp32 precision (free dim 256 >= the full-rate cutoff)
                lhs = [x1t_sb[0], x1t_sb[1], xt_sb[0], xt_sb[1]]
                for k in range(3):
                    po = psum2_pool.tile([128, O], f32, tag="po")
                    for fc in range(FCH):
                        nc.tensor.matmul(
                            po[:],
                            lhs[fc][:, jcols],
                            w_sb[k][fc][:],
                            start=(fc == 0),
                            stop=(fc == FCH - 1),
                        )
                    ot = work_pool.tile([128, O], f32, tag="ot")
                    if with_bias:
                        nc.vector.tensor_add(ot[:], po[:], bias_sb[:])
                    else:
                        nc.vector.tensor_copy(ot[:], po[:])
                    nc.scalar.dma_start(out[k, j * 128:(j + 1) * 128, :], ot[:])

            # software-pipeline by one node-tile so PE never stalls on the
            # DVE reciprocal/divide between stage-1 accumulation and stage 2
            prev = (0, ps0)
            for j in range(1, JT):
                mt = load_mask(j)
                ps = stage1(j, mt)
                finalize(*prev)
                prev = (j, ps)
            finalize(*prev)

    return nc


_cached = {}


def _get_bass(with_bias: bool):
    if with_bias not in _cached:
        _cached[with_bias] = build_bass(with_bias)
    return _cached[with_bias]


def _host_prep(x, adj, weight, bias):
    import ml_dtypes

    mask_np = ml_dtypes.bfloat16 if _MASK_BF16 else np.float32
    x = np.asarray(x, dtype=np.float32)
    adj = np.asarray(adj)
    weight = np.asarray(weight, dtype=np.float32)
    bias = np.asarray(bias, dtype=np.float32)

    with_bias = bool(np.any(bias))

    # replicated: [x | 1] in stage-1 layout [128 p][chunk c][F+1]
    xp = np.empty((N, FP), dtype=np.float32)
    xp[:, :F] = x
    xp[:, F] = 1.0
    xp_t = np.ascontiguousarray(
        xp.reshape(MCHUNKS, 128, FP).transpose(1, 0, 2)
    ).reshape(128, MCHUNKS * FP).astype(mask_np)

    w_t = np.ascontiguousarray(weight.reshape(3, 2 * F // 128, 128, O))
    bias_r = np.broadcast_to(bias, (128, O)).copy() if with_bias else None

    mask = (adj == 1)
    in_maps = []
    for c in range(NCORES):
        rows = slice(c * NB, (c + 1) * NB)
        # adjT shard in DMA-friendly layout [j][p][g c n]: element
        # [j, p, g*1024 + ci*128 + n] = mask[node j*128+n, m=g*1024+ci*128+p]
        a = mask[rows].T.astype(mask_np)          # [N m, NB n]
        a = a.reshape(MCHUNKS // 8, 8, 128, JT, 128)   # [g, ci, p, j, n]
        a = np.ascontiguousarray(a.transpose(3, 2, 0, 1, 4)).reshape(
            JT, 128, MCHUNKS * 128
        )
        xt_c = np.ascontiguousarray(x[rows].T).reshape(F // 128, 128, NB)
        m = {"maskt": a, "xp": xp_t, "xt": xt_c, "w": w_t}
        if with_bias:
            m["biasr"] = bias_r
        in_maps.append(m)
    return in_maps, with_bias


def run(x, adj, weight, bias, trace=False, trace_kwargs=None):
    """Shard, run on 8 cores, gather. Returns (out_full, BassKernelResults)."""
    from concourse.bass_utils import run_bass_kernel_spmd

    in_maps, with_bias = _host_prep(x, adj, weight, bias)
    nc = _get_bass(with_bias)
    res = run_bass_kernel_spmd(
        nc, in_maps, list(range(NCORES)), trace=trace, **(trace_kwargs or {})
    )
    out_full = np.empty((3, N, O), dtype=np.float32)
    for c in range(NCORES):
        out_full[:, c * NB:(c + 1) * NB, :] = res.results[c]["out"]
    return out_full, res


def kernel(g, x, adj, weight, bias):
    out, _ = run(x, adj, weight, bias)
    return out



# revision 3
# speedup vs baseline: 1.3338x; 1.3338x over previous
"""Trainium2 Bass kernel for CoreSageLayer (GNN mean-aggregate + 3-way linear).

Computation (reference):
    mask = (adj == 1)                      # [N, N] 0/1
    deg  = mask.sum(axis=1)                # [N]
    x1   = (mask @ x) / deg[:, None]       # [N, F]
    out[k] = concat([x1, x], 1) @ W[k] + bias   # [3, N, O]

Distribution: row-shard adj / x1 / out over nodes across 8 cores; replicate
x and weights; no collectives (rows independent).

Device schedule per core (nodes NB=1024, 8 node-tiles of 128):
  stage 1 (per node-tile j): fp8e4m3 DoubleRow matmuls — each instruction
      contracts a 256-neighbor chunk-pair (2 fp8 rows per PE cell, 0.5
      cycles per output column). lhsT = maskT chunk [128, 2, 128], rhs =
      [1 | x] fp8 chunk. Column 0 of the moving tensor is the ones column,
      so PSUM col 0 accumulates the exact degree (0/1 products in fp32).
      Output columns split 129 + 128 across two PSUM tiles because the
      DoubleRow moving limit is 512 (2*257 = 514 would exceed it).
  finalize j: rec = 1/deg (DVE), x1 = psum * rec -> bf16, PE-transpose x1
      into x1T (bf16), then stage 2: out[k] = [x1 | x] @ W[k] as bf16
      matmuls with W for k=0,1 fused into one 512-wide moving tensor.
      Results are written back as bf16 (cast to f32 on host).
"""

import sys

sys.path.insert(0, "/opt/trn_rl_repo")

import numpy as np

N = 8192
F = 256
O = 256
NCORES = 8
NB = N // NCORES          # nodes per core (1024)
JT = NB // 128            # node tiles per core (8)
MCHUNKS = N // 128        # contraction chunks of 128 (64)
CP = MCHUNKS // 2         # chunk-pairs of 256 for DoubleRow (32)
FP = F + 1                # ones column + x columns (257)


def _patch_tile_drain():
    """This container's walrus allows only one sync-wait per CTRL instruction;
    split the Tile kernel-tail drain's waits onto single-wait no-fuse NoOps."""
    import concourse.tile as tile
    from concourse import mybir
    from concourse.tile import ScopedClock

    if getattr(tile.TileContext, "_drain_split_patched", False):
        return

    def _drain_and_barrier(self, tick_clock, wait_clock):
        nc = self.nc
        drain_inst = nc.sync.drain()
        wait_clock.add_sem_waits(
            drain_inst.ins, ScopedClock({None: tick_clock.global_clock})
        )
        si = drain_inst.ins.sync_info
        if si is not None and len(si.on_wait) > 1:
            waits = list(si.on_wait)
            drain_inst.ins.sync_info = mybir.SyncInfo(
                on_wait=[waits[0]], on_update=list(si.on_update)
            )
            for w in waits[1:]:
                nop = nc.sync.nop(nofuse=True, hint="split_wait")
                nop.ins.sync_info = mybir.SyncInfo(on_wait=[w], on_update=[])
        nc.all_engine_barrier()
        assert self.sems is not None
        popped = nc._tile_sem_poison_stack.pop()
        assert popped is self._sem_poison
        nc.clear_and_free_semaphores(list(self.sems.allocated().values()))
        nc.all_engine_barrier()

    tile.TileContext._drain_and_barrier = _drain_and_barrier
    tile.TileContext._drain_split_patched = True

    # Same walrus limitation, general case: any instruction that Tile gave
    # >1 sem-wait (e.g. a DMA with both RAW and WAR deps) fails codegen.
    # Split surplus waits onto fresh single-wait NoOps emitted just before
    # the instruction on the same engine, at the serialized-BIR level.
    import concourse.bass as bass
    import orjson

    _orig_to_json_bytes = bass.Bass.to_json_bytes

    def _to_json_bytes_split(self):
        m = orjson.loads(_orig_to_json_bytes(self))
        ctr = 0
        for fn in m.get("functions", []):
            for bb in fn.get("blocks", []):
                insts = bb.get("instructions", [])
                new = []
                for inst in insts:
                    si = inst.get("sync_info")
                    waits = (si or {}).get("on_wait") or []
                    if len(waits) > 1:
                        for w in waits[:-1]:
                            ctr += 1
                            new.append({
                                "name": f"SWNOP-{ctr}",
                                "opcode": "NoOp",
                                "engine": inst["engine"],
                                "ins": [],
                                "outs": [],
                                "sync_info": {"on_wait": [w], "on_update": []},
                            })
                        si["on_wait"] = [waits[-1]]
                    new.append(inst)
                bb["instructions"] = new
        return orjson.dumps(m)

    bass.Bass.to_json_bytes = _to_json_bytes_split


def build_bass(with_bias: bool):
    import concourse.bass as bass
    import concourse.tile as tile
    from concourse import mybir
    from concourse.masks import make_identity

    _patch_tile_drain()

    fp8 = mybir.dt.float8e4
    bf16 = mybir.dt.bfloat16
    f32 = mybir.dt.float32
    DR = mybir.MatmulPerfMode.DoubleRow

    nc = bass.Bass()
    maskt = nc.dram_tensor("maskt", [JT, 128, MCHUNKS * 128], fp8,
                           kind="ExternalInput")
    xp = nc.dram_tensor("xp", [128, MCHUNKS * FP], fp8, kind="ExternalInput")
    xt = nc.dram_tensor("xt", [F // 128, 128, NB], bf16, kind="ExternalInput")
    # W for k=0,1 fused along the output dim (512-wide moving tensor); k=2 alone
    w01 = nc.dram_tensor("w01", [2 * F // 128, 128, 2 * O], bf16,
                         kind="ExternalInput")
    w2 = nc.dram_tensor("w2", [2 * F // 128, 128, O], bf16, kind="ExternalInput")
    if with_bias:
        biasr = nc.dram_tensor("biasr", [128, O], f32, kind="ExternalInput")
    out = nc.dram_tensor("out", [3, NB, O], bf16, kind="ExternalOutput")

    FCH = 2 * F // 128  # 4 f-chunks of 128 in the stage-2 contraction

    with tile.TileContext(nc) as tc:
        with (
            tc.tile_pool(name="const", bufs=1) as const_pool,
            tc.tile_pool(name="mask", bufs=3) as mask_pool,
            tc.tile_pool(name="work", bufs=3) as work_pool,
            tc.tile_pool(name="psumA", bufs=2, space="PSUM") as psumA_pool,
            tc.tile_pool(name="psumB", bufs=2, space="PSUM") as psumB_pool,
            tc.tile_pool(name="psumt", bufs=2, space="PSUM") as psumt_pool,
            tc.tile_pool(name="psum2", bufs=2, space="PSUM") as psum2_pool,
        ):
            # ---- DMA order matters: everything shares the HWDGE FIFO.
            # First the j=0 mask block interleaved with [1|x] pieces so the
            # PE unblocks at stream rate, then stage-2 constants.
            mt0 = mask_pool.tile([128, MCHUNKS, 128], fp8, tag="mt", name="mt0")
            xp_sb = const_pool.tile([128, MCHUNKS, FP], fp8)
            MSPL, XSPL = 4, 8
            mw = MCHUNKS // MSPL
            xw = MCHUNKS // XSPL
            for q in range(MSPL):
                nc.sync.dma_start(mt0[:, q * mw:(q + 1) * mw, :],
                                  maskt[0, :, q * mw * 128:(q + 1) * mw * 128])
                for xq in (2 * q, 2 * q + 1):
                    nc.sync.dma_start(
                        xp_sb[:, xq * xw:(xq + 1) * xw, :],
                        xp[:, xq * xw * FP:(xq + 1) * xw * FP])

            def stage1(j, mt):
                psA = psumA_pool.tile([128, 129], f32, tag="psA")
                psB = psumB_pool.tile([128, 128], f32, tag="psB")
                for c in range(CP):
                    lhsT = mt[:, 2 * c:2 * c + 2, :]
                    nc.tensor.matmul(
                        psA[:], lhsT, xp_sb[:, 2 * c:2 * c + 2, 0:129],
                        start=(c == 0), stop=(c == CP - 1), perf_mode=DR,
                    )
                    nc.tensor.matmul(
                        psB[:], lhsT, xp_sb[:, 2 * c:2 * c + 2, 129:257],
                        start=(c == 0), stop=(c == CP - 1), perf_mode=DR,
                    )
                return psA, psB

            ps0 = stage1(0, mt0)

            # stage-2 constants: emitted after stage1(0), used by finalize(0)
            xt_sb = [const_pool.tile([128, NB], bf16, tag=f"xt{h}", name=f"xt{h}")
                     for h in range(2)]
            for h in range(2):
                nc.scalar.dma_start(xt_sb[h][:], xt[h])
            w01_sb = [const_pool.tile([128, 2 * O], bf16, tag=f"w01_{fc}",
                                      name=f"w01_{fc}") for fc in range(FCH)]
            w2_sb = [const_pool.tile([128, O], bf16, tag=f"w2_{fc}",
                                     name=f"w2_{fc}") for fc in range(FCH)]
            for fc in range(FCH):
                nc.scalar.dma_start(w01_sb[fc][:], w01[fc])
            for fc in range(FCH):
                nc.scalar.dma_start(w2_sb[fc][:], w2[fc])
            if with_bias:
                bias_sb = const_pool.tile([128, O], f32)
                nc.scalar.dma_start(bias_sb[:], biasr[:])
            identity = const_pool.tile([128, 128], bf16)
            make_identity(nc, identity)
            # x1T[h] row f (= h*128+f), col n: x1 transposed, filled per j
            x1t_sb = [const_pool.tile([128, NB], bf16, tag=f"x1t{h}",
                                      name=f"x1t{h}") for h in range(2)]

            def load_mask(j):
                mt = mask_pool.tile([128, MCHUNKS, 128], fp8, tag="mt",
                                    name=f"mt{j}")
                nc.sync.dma_start(mt[:], maskt[j])
                return mt

            def finalize(j, psA, psB):
                jcols = slice(j * 128, (j + 1) * 128)
                jrows = slice(j * 128, (j + 1) * 128)
                rec = work_pool.tile([128, 1], f32, tag="rec")
                nc.vector.reciprocal(rec[:], psA[:, 0:1])
                x1h = [work_pool.tile([128, 128], bf16, tag=f"x1h{h}",
                                      name=f"x1h{h}") for h in range(2)]
                nc.vector.tensor_scalar_mul(x1h[0][:], psA[:, 1:129], rec[:])
                nc.vector.tensor_scalar_mul(x1h[1][:], psB[:], rec[:])
                for h in range(2):
                    pt = psumt_pool.tile([128, 128], bf16, tag="pt")
                    nc.tensor.transpose(pt[:], x1h[h][:], identity[:])
                    nc.vector.tensor_copy(x1t_sb[h][:, jcols], pt[:])
                lhs = [x1t_sb[0], x1t_sb[1], xt_sb[0], xt_sb[1]]
                po01 = psum2_pool.tile([128, 2 * O], f32, tag="po")
                for fc in range(FCH):
                    nc.tensor.matmul(
                        po01[:], lhs[fc][:, jcols], w01_sb[fc][:],
                        start=(fc == 0), stop=(fc == FCH - 1),
                    )
                ot01 = work_pool.tile([128, 2 * O], bf16, tag="ot01")
                if with_bias:
                    nc.vector.tensor_add(ot01[:, 0:O], po01[:, 0:O], bias_sb[:])
                    nc.vector.tensor_add(ot01[:, O:2 * O], po01[:, O:2 * O],
                                         bias_sb[:])
                else:
                    nc.vector.tensor_copy(ot01[:], po01[:])
                nc.scalar.dma_start(out[0, jrows, :], ot01[:, 0:O])
                nc.scalar.dma_start(out[1, jrows, :], ot01[:, O:2 * O])
                po2 = psum2_pool.tile([128, 2 * O], f32, tag="po")
                for fc in range(FCH):
                    nc.tensor.matmul(
                        po2[:, 0:O], lhs[fc][:, jcols], w2_sb[fc][:],
                        start=(fc == 0), stop=(fc == FCH - 1),
                    )
                ot2 = work_pool.tile([128, O], bf16, tag="ot2")
                if with_bias:
                    nc.vector.tensor_add(ot2[:], po2[:, 0:O], bias_sb[:])
                else:
                    nc.vector.tensor_copy(ot2[:], po2[:, 0:O])
                nc.scalar.dma_start(out[2, jrows, :], ot2[:])

            # software-pipeline by one node-tile so PE never stalls on the
            # DVE reciprocal/divide between stage-1 accumulation and stage 2
            prev = (0, *ps0)
            for j in range(1, JT):
                mt = load_mask(j)
                ps = stage1(j, mt)
                finalize(*prev)
                prev = (j, *ps)
            finalize(*prev)

    return nc


_cached = {}


def _get_bass(with_bias: bool):
    if with_bias not in _cached:
        _cached[with_bias] = build_bass(with_bias)
    return _cached[with_bias]


def _host_prep(x, adj, weight, bias):
    import ml_dtypes

    fp8 = ml_dtypes.float8_e4m3
    bf16 = ml_dtypes.bfloat16
    x = np.asarray(x, dtype=np.float32)
    adj = np.asarray(adj)
    weight = np.asarray(weight, dtype=np.float32)
    bias = np.asarray(bias, dtype=np.float32)

    with_bias = bool(np.any(bias))

    # replicated: [1 | x] in stage-1 layout [128 p][chunk c][1+F], fp8
    xpf = np.empty((N, FP), dtype=np.float32)
    xpf[:, 0] = 1.0
    xpf[:, 1:] = x
    xp_t = np.ascontiguousarray(
        xpf.reshape(MCHUNKS, 128, FP).transpose(1, 0, 2)
    ).reshape(128, MCHUNKS * FP).astype(fp8)

    # stage-2 weights: k=0,1 fused along output dim; k=2 separate (bf16)
    w_r = weight.reshape(3, 2 * F // 128, 128, O)
    w01_t = np.ascontiguousarray(
        w_r[0:2].transpose(1, 2, 0, 3)
    ).reshape(2 * F // 128, 128, 2 * O).astype(bf16)
    w2_t = np.ascontiguousarray(w_r[2]).astype(bf16)
    bias_r = np.broadcast_to(bias, (128, O)).copy() if with_bias else None

    mask = (adj == 1)
    in_maps = []
    for c in range(NCORES):
        rows = slice(c * NB, (c + 1) * NB)
        # adjT shard in layout [j][p][c][n]: element
        # [j, p, c*128 + n] = mask[node j*128+n, m=c*128+p], fp8
        a = mask[rows].T.astype(fp8)                   # [N m, NB n]
        a = a.reshape(MCHUNKS, 128, JT, 128)           # [c, p, j, n]
        a = np.ascontiguousarray(a.transpose(2, 1, 0, 3)).reshape(
            JT, 128, MCHUNKS * 128
        )
        xt_c = np.ascontiguousarray(x[rows].T).reshape(F // 128, 128, NB
                                                       ).astype(bf16)
        m = {"maskt": a, "xp": xp_t, "xt": xt_c, "w01": w01_t, "w2": w2_t}
        if with_bias:
            m["biasr"] = bias_r
        in_maps.append(m)
    return in_maps, with_bias


def run(x, adj, weight, bias, trace=False, trace_kwargs=None):
    """Shard, run on 8 cores, gather. Returns (out_full, BassKernelResults)."""
    from concourse.bass_utils import run_bass_kernel_spmd

    in_maps, with_bias = _host_prep(x, adj, weight, bias)
    nc = _get_bass(with_bias)
    res = run_bass_kernel_spmd(
        nc, in_maps, list(range(NCORES)), trace=trace, **(trace_kwargs or {})
    )
    out_full = np.empty((3, N, O), dtype=np.float32)
    for c in range(NCORES):
        out_full[:, c * NB:(c + 1) * NB, :] = np.asarray(
            res.results[c]["out"], dtype=np.float32
        )
    return out_full, res


def kernel(g, x, adj, weight, bias):
    out, _ = run(x, adj, weight, bias)
    return out


# revision 5
# speedup vs baseline: 1.4396x; 1.0793x over previous
"""Trainium2 Bass kernel for CoreSageLayer (GNN mean-aggregate + 3-way linear).

Computation (reference):
    mask = (adj == 1)                      # [N, N] 0/1
    deg  = mask.sum(axis=1)                # [N]
    x1   = (mask @ x) / deg[:, None]       # [N, F]
    out[k] = concat([x1, x], 1) @ W[k] + bias   # [3, N, O]

Distribution: row-shard adj / x1 / out over nodes across 8 cores; replicate
x and weights; no collectives (rows independent).

Device schedule per core (nodes NB=1024, 8 node-tiles of 128):
  stage 1 (per node-tile j): fp8e4m3 DoubleRow matmuls — each instruction
      contracts a 256-neighbor chunk-pair (2 fp8 rows per PE cell, 0.5
      cycles per output column). lhsT = maskT chunk [128, 2, 128], rhs =
      [1 | x] fp8 chunk. Column 0 of the moving tensor is the ones column,
      so PSUM col 0 accumulates the exact degree (0/1 products in fp32).
      Output columns split 129 + 128 across two PSUM tiles because the
      DoubleRow moving limit is 512 (2*257 = 514 would exceed it).
  finalize j: rec = 1/deg (DVE), x1 = psum * rec -> bf16, PE-transpose x1
      into x1T (bf16), then stage 2: out[k] = [x1 | x] @ W[k] as bf16
      matmuls with W for k=0,1 fused into one 512-wide moving tensor.
      Results are written back as bf16 (cast to f32 on host).
"""

import sys

sys.path.insert(0, "/opt/trn_rl_repo")

import numpy as np

N = 8192
F = 256
O = 256
NCORES = 8
NB = N // NCORES          # nodes per core (1024)
JT = NB // 128            # node tiles per core (8)
MCHUNKS = N // 128        # contraction chunks of 128 (64)
CP = MCHUNKS // 2         # chunk-pairs of 256 for DoubleRow (32)
FP = F + 1                # ones column + x columns (257)


def _patch_tile_drain():
    """This container's walrus allows only one sync-wait per CTRL instruction;
    split the Tile kernel-tail drain's waits onto single-wait no-fuse NoOps."""
    import concourse.tile as tile
    from concourse import mybir
    from concourse.tile import ScopedClock

    if getattr(tile.TileContext, "_drain_split_patched", False):
        return

    def _drain_and_barrier(self, tick_clock, wait_clock):
        nc = self.nc
        drain_inst = nc.sync.drain()
        wait_clock.add_sem_waits(
            drain_inst.ins, ScopedClock({None: tick_clock.global_clock})
        )
        si = drain_inst.ins.sync_info
        if si is not None and len(si.on_wait) > 1:
            waits = list(si.on_wait)
            drain_inst.ins.sync_info = mybir.SyncInfo(
                on_wait=[waits[0]], on_update=list(si.on_update)
            )
            for w in waits[1:]:
                nop = nc.sync.nop(nofuse=True, hint="split_wait")
                nop.ins.sync_info = mybir.SyncInfo(on_wait=[w], on_update=[])
        nc.all_engine_barrier()
        assert self.sems is not None
        popped = nc._tile_sem_poison_stack.pop()
        assert popped is self._sem_poison
        nc.clear_and_free_semaphores(list(self.sems.allocated().values()))
        nc.all_engine_barrier()

    tile.TileContext._drain_and_barrier = _drain_and_barrier
    tile.TileContext._drain_split_patched = True

    # Same walrus limitation, general case: any instruction that Tile gave
    # >1 sem-wait (e.g. a DMA with both RAW and WAR deps) fails codegen.
    # Split surplus waits onto fresh single-wait NoOps emitted just before
    # the instruction on the same engine, at the serialized-BIR level.
    import concourse.bass as bass
    import orjson

    _orig_to_json_bytes = bass.Bass.to_json_bytes

    def _to_json_bytes_split(self):
        m = orjson.loads(_orig_to_json_bytes(self))
        ctr = 0
        for fn in m.get("functions", []):
            for bb in fn.get("blocks", []):
                insts = bb.get("instructions", [])
                # Dedupe redundant PE weight loads: legalization emits one
                # Ldweights per Matmult, but consecutive matmuls that share
                # a stationary tensor (the two output-column groups per mask
                # chunk-pair; the three k's per stage-2 f-chunk) only need
                # the first — the PE array keeps weights across matmuls.
                # A duplicate with sync waits/updates becomes a NoOp that
                # preserves them; a bare one is dropped.
                deduped = []
                cur_key = None
                for inst in insts:
                    if inst.get("engine") != "PE":
                        deduped.append(inst)
                        continue
                    op = inst.get("opcode")
                    if op == "Ldweights":
                        key = orjson.dumps([
                            inst.get("ins"), inst.get("perf_mode"),
                            inst.get("is_transpose"),
                            inst.get("tile_position"), inst.get("tile_size"),
                        ])
                        if key == cur_key:
                            si = inst.get("sync_info")
                            if si and (si.get("on_wait") or si.get("on_update")):
                                deduped.append({
                                    "name": inst["name"] + "-LDWNOP",
                                    "opcode": "NoOp",
                                    "engine": "PE",
                                    "ins": [],
                                    "outs": [],
                                    "sync_info": si,
                                })
                            continue
                        cur_key = key
                    elif op != "Matmult":
                        cur_key = None
                    deduped.append(inst)
                insts = deduped
                new = []
                for inst in insts:
                    si = inst.get("sync_info")
                    waits = (si or {}).get("on_wait") or []
                    if len(waits) > 1:
                        for w in waits[:-1]:
                            ctr += 1
                            new.append({
                                "name": f"SWNOP-{ctr}",
                                "opcode": "NoOp",
                                "engine": inst["engine"],
                                "ins": [],
                                "outs": [],
                                "sync_info": {"on_wait": [w], "on_update": []},
                            })
                        si["on_wait"] = [waits[-1]]
                    new.append(inst)
                bb["instructions"] = new
        return orjson.dumps(m)

    bass.Bass.to_json_bytes = _to_json_bytes_split



def build_bass(with_bias: bool):
    import concourse.bass as bass
    import concourse.tile as tile
    from concourse import mybir
    from concourse.masks import make_identity

    _patch_tile_drain()

    fp8 = mybir.dt.float8e4
    bf16 = mybir.dt.bfloat16
    f32 = mybir.dt.float32
    DR = mybir.MatmulPerfMode.DoubleRow

    nc = bass.Bass()
    maskt = nc.dram_tensor("maskt", [JT, 128, MCHUNKS * 128], fp8,
                           kind="ExternalInput")
    xp = nc.dram_tensor("xp", [128, MCHUNKS * FP], fp8, kind="ExternalInput")
    xt = nc.dram_tensor("xt", [F // 128, 128, NB], bf16, kind="ExternalInput")
    # W for k=0,1 fused along the output dim (512-wide moving tensor); k=2 alone
    w01 = nc.dram_tensor("w01", [2 * F // 128, 128, 2 * O], bf16,
                         kind="ExternalInput")
    w2 = nc.dram_tensor("w2", [2 * F // 128, 128, O], bf16, kind="ExternalInput")
    if with_bias:
        biasr = nc.dram_tensor("biasr", [128, O], f32, kind="ExternalInput")
    out = nc.dram_tensor("out", [JT, 128, 3 * O], bf16, kind="ExternalOutput")

    FCH = 2 * F // 128  # 4 f-chunks of 128 in the stage-2 contraction

    with tile.TileContext(nc) as tc:
        with (
            tc.tile_pool(name="const", bufs=1) as const_pool,
            tc.tile_pool(name="mask", bufs=3) as mask_pool,
            tc.tile_pool(name="work", bufs=3) as work_pool,
            tc.tile_pool(name="psumA", bufs=2, space="PSUM") as psumA_pool,
            tc.tile_pool(name="psumB", bufs=2, space="PSUM") as psumB_pool,
            tc.tile_pool(name="psumt", bufs=2, space="PSUM") as psumt_pool,
            tc.tile_pool(name="psum2", bufs=2, space="PSUM") as psum2_pool,
        ):
            # ---- DMA order matters: everything shares the HWDGE FIFO.
            # First the j=0 mask block interleaved with [1|x] pieces so the
            # PE unblocks at stream rate, then stage-2 constants.
            mt0 = mask_pool.tile([128, MCHUNKS, 128], fp8, tag="mt", name="mt0")
            xp_sb = const_pool.tile([128, MCHUNKS, FP], fp8)
            MSPL, XSPL = 4, 8
            mw = MCHUNKS // MSPL
            xw = MCHUNKS // XSPL
            for q in range(MSPL):
                nc.sync.dma_start(mt0[:, q * mw:(q + 1) * mw, :],
                                  maskt[0, :, q * mw * 128:(q + 1) * mw * 128])
                for xq in (2 * q, 2 * q + 1):
                    nc.sync.dma_start(
                        xp_sb[:, xq * xw:(xq + 1) * xw, :],
                        xp[:, xq * xw * FP:(xq + 1) * xw * FP])

            def stage1(j, mt):
                psA = psumA_pool.tile([128, 129], f32, tag="psA")
                psB = psumB_pool.tile([128, 128], f32, tag="psB")
                for c in range(CP):
                    lhsT = mt[:, 2 * c:2 * c + 2, :]
                    nc.tensor.matmul(
                        psA[:], lhsT, xp_sb[:, 2 * c:2 * c + 2, 0:129],
                        start=(c == 0), stop=(c == CP - 1), perf_mode=DR,
                    )
                    nc.tensor.matmul(
                        psB[:], lhsT, xp_sb[:, 2 * c:2 * c + 2, 129:257],
                        start=(c == 0), stop=(c == CP - 1), perf_mode=DR,
                    )
                return psA, psB

            ps0 = stage1(0, mt0)

            # stage-2 constants: emitted after stage1(0), used by finalize(0)
            xt_sb = [const_pool.tile([128, NB], bf16, tag=f"xt{h}", name=f"xt{h}")
                     for h in range(2)]
            for h in range(2):
                nc.scalar.dma_start(xt_sb[h][:], xt[h])
            w01_sb = [const_pool.tile([128, 2 * O], bf16, tag=f"w01_{fc}",
                                      name=f"w01_{fc}") for fc in range(FCH)]
            w2_sb = [const_pool.tile([128, O], bf16, tag=f"w2_{fc}",
                                     name=f"w2_{fc}") for fc in range(FCH)]
            for fc in range(FCH):
                nc.scalar.dma_start(w01_sb[fc][:], w01[fc])
            for fc in range(FCH):
                nc.scalar.dma_start(w2_sb[fc][:], w2[fc])
            if with_bias:
                bias_sb = const_pool.tile([128, O], f32)
                nc.scalar.dma_start(bias_sb[:], biasr[:])
            identity = const_pool.tile([128, 128], bf16)
            make_identity(nc, identity)
            # x1T[h] row f (= h*128+f), col n: x1 transposed, filled per j
            x1t_sb = [const_pool.tile([128, NB], bf16, tag=f"x1t{h}",
                                      name=f"x1t{h}") for h in range(2)]

            def load_mask(j):
                mt = mask_pool.tile([128, MCHUNKS, 128], fp8, tag="mt",
                                    name=f"mt{j}")
                nc.sync.dma_start(mt[:], maskt[j])
                return mt

            def finalize(j, psA, psB):
                jcols = slice(j * 128, (j + 1) * 128)
                rec = work_pool.tile([128, 1], f32, tag="rec")
                nc.vector.reciprocal(rec[:], psA[:, 0:1])
                x1h = [work_pool.tile([128, 128], bf16, tag=f"x1h{h}",
                                      name=f"x1h{h}") for h in range(2)]
                nc.vector.tensor_scalar_mul(x1h[0][:], psA[:, 1:129], rec[:])
                nc.vector.tensor_scalar_mul(x1h[1][:], psB[:], rec[:])
                for h in range(2):
                    pt = psumt_pool.tile([128, 128], bf16, tag="pt")
                    nc.tensor.transpose(pt[:], x1h[h][:], identity[:])
                    nc.vector.tensor_copy(x1t_sb[h][:, jcols], pt[:])
                lhs = [x1t_sb[0], x1t_sb[1], xt_sb[0], xt_sb[1]]
                po01 = psum2_pool.tile([128, 2 * O], f32, tag="po")
                po2 = psum2_pool.tile([128, 2 * O], f32, tag="po")
                for fc in range(FCH):
                    nc.tensor.matmul(
                        po01[:], lhs[fc][:, jcols], w01_sb[fc][:],
                        start=(fc == 0), stop=(fc == FCH - 1),
                    )
                    nc.tensor.matmul(
                        po2[:, 0:O], lhs[fc][:, jcols], w2_sb[fc][:],
                        start=(fc == 0), stop=(fc == FCH - 1),
                    )
                ot = work_pool.tile([128, 3 * O], bf16, tag="ot")
                if with_bias:
                    nc.vector.tensor_add(ot[:, 0:O], po01[:, 0:O], bias_sb[:])
                    nc.vector.tensor_add(ot[:, O:2 * O], po01[:, O:2 * O],
                                         bias_sb[:])
                    nc.vector.tensor_add(ot[:, 2 * O:], po2[:, 0:O], bias_sb[:])
                else:
                    nc.vector.tensor_copy(ot[:, 0:2 * O], po01[:])
                    nc.vector.tensor_copy(ot[:, 2 * O:], po2[:, 0:O])
                nc.gpsimd.dma_start(out[j], ot[:])

            # software-pipeline by one node-tile so PE never stalls on the
            # DVE reciprocal/divide between stage-1 accumulation and stage 2
            prev = (0, *ps0)
            for j in range(1, JT):
                mt = load_mask(j)
                ps = stage1(j, mt)
                finalize(*prev)
                prev = (j, *ps)
            finalize(*prev)

    return nc


_cached = {}


def _get_bass(with_bias: bool):
    if with_bias not in _cached:
        _cached[with_bias] = build_bass(with_bias)
    return _cached[with_bias]


def _host_prep(x, adj, weight, bias):
    import ml_dtypes

    fp8 = ml_dtypes.float8_e4m3
    bf16 = ml_dtypes.bfloat16
    x = np.asarray(x, dtype=np.float32)
    adj = np.asarray(adj)
    weight = np.asarray(weight, dtype=np.float32)
    bias = np.asarray(bias, dtype=np.float32)

    with_bias = bool(np.any(bias))

    # replicated: [1 | x] in stage-1 layout [128 p][chunk c][1+F], fp8
    xpf = np.empty((N, FP), dtype=np.float32)
    xpf[:, 0] = 1.0
    xpf[:, 1:] = x
    xp_t = np.ascontiguousarray(
        xpf.reshape(MCHUNKS, 128, FP).transpose(1, 0, 2)
    ).reshape(128, MCHUNKS * FP).astype(fp8)

    # stage-2 weights: k=0,1 fused along output dim; k=2 separate (bf16)
    w_r = weight.reshape(3, 2 * F // 128, 128, O)
    w01_t = np.ascontiguousarray(
        w_r[0:2].transpose(1, 2, 0, 3)
    ).reshape(2 * F // 128, 128, 2 * O).astype(bf16)
    w2_t = np.ascontiguousarray(w_r[2]).astype(bf16)
    bias_r = np.broadcast_to(bias, (128, O)).copy() if with_bias else None

    mask = (adj == 1)
    in_maps = []
    for c in range(NCORES):
        rows = slice(c * NB, (c + 1) * NB)
        # adjT shard in layout [j][p][c][n]: element
        # [j, p, c*128 + n] = mask[node j*128+n, m=c*128+p], fp8
        a = mask[rows].T.astype(fp8)                   # [N m, NB n]
        a = a.reshape(MCHUNKS, 128, JT, 128)           # [c, p, j, n]
        a = np.ascontiguousarray(a.transpose(2, 1, 0, 3)).reshape(
            JT, 128, MCHUNKS * 128
        )
        xt_c = np.ascontiguousarray(x[rows].T).reshape(F // 128, 128, NB
                                                       ).astype(bf16)
        m = {"maskt": a, "xp": xp_t, "xt": xt_c, "w01": w01_t, "w2": w2_t}
        if with_bias:
            m["biasr"] = bias_r
        in_maps.append(m)
    return in_maps, with_bias


def run(x, adj, weight, bias, trace=False, trace_kwargs=None):
    """Shard, run on 8 cores, gather. Returns (out_full, BassKernelResults)."""
    from concourse.bass_utils import run_bass_kernel_spmd

    in_maps, with_bias = _host_prep(x, adj, weight, bias)
    nc = _get_bass(with_bias)
    res = run_bass_kernel_spmd(
        nc, in_maps, list(range(NCORES)), trace=trace, **(trace_kwargs or {})
    )
    out_full = np.empty((3, N, O), dtype=np.float32)
    for c in range(NCORES):
        o = np.asarray(res.results[c]["out"], dtype=np.float32)
        out_full[:, c * NB:(c + 1) * NB, :] = o.reshape(
            NB, 3, O).transpose(1, 0, 2)
    return out_full, res


def kernel(g, x, adj, weight, bias):
    out, _ = run(x, adj, weight, bias)
    return out
